# revision 25
# baseline (speedup 1.0000x reference)
"""GATv2Conv batched-graph kernel for Trainium2 (8 NeuronCores, data-parallel).

Problem: B=16384 independent 9-node graphs, C_in=C_out=256, fixed edge list
(16 directed tree edges + 9 self-loops = 25 edges), GATv2 attention.

V2 design (vs the 342us baseline):
  - Cross-block software pipelining: the PE stream for block b interleaves
    block b's projections (always-ready work) with block b-1's score
    matmuls (gated on DVE-made leaky tiles).  This keeps the tensor engine
    continuously busy so it ramps to its full 2.4 GHz p-state (a cold/idle
    PE runs at 1.2 GHz).
  - Edges are ordered self-loops-first then grouped by source node, so the
    per-edge adds+leaky-relu collapse into a few wide strided ops
    (self-loops: one [128, 9*G] op; per-src groups: dst slices with a
    uniform node stride).  Leaky is applied in place on the h tiles.
  - The leaky tiles (score path only) are stored as fp8-e4m3: the score is
    a 256-term dot product, so the elementwise quantization error averages
    out (~0.4% on scores, well inside the 2e-2 budget).
  - True leaky-relu in one op per tile (ACT Prelu / DVE-Pool STT
    max(0.2x, x)); the old separate 0.2*x linear score path (nu/vsel) is
    gone.
  - Aggregation in graph-major via fused scalar_tensor_tensor chains
    (alpha is a per-partition scalar there), fed by a second graph-major
    xl projection (bf16 - it feeds the output directly).
  - bias is handled host-side (zeros in this problem).
"""

import sys

if "/opt/trn_rl_repo" not in sys.path:
    sys.path.insert(0, "/opt/trn_rl_repo")

import numpy as np
import ml_dtypes

import concourse.bass as bass
import concourse.bacc as bacc
import concourse.mybir as mybir
from concourse import tile
from concourse.bass_utils import run_bass_kernel_spmd

F32 = mybir.dt.float32
BF16 = mybir.dt.bfloat16
FP8 = mybir.dt.bfloat16  # st dtype (fp8 fails tolerance)

N_CORES = 8
B_TOTAL = 16384
NEG_SLOPE = 0.2
BC = B_TOTAL // N_CORES          # graphs per core
NN = 9                           # nodes per graph
C = 256                          # channels
G = 256                          # graphs per block
NBLK = BC // G                   # blocks per core
NT = G // 128                    # 128-graph subtiles per block
NGT = NN * G                     # columns per (chunk, block)

# ---- static edge list ----
# Order: 9 self-loops first (edge e = node e), then tree edges grouped by
# SOURCE node, with the dst list of each group an arithmetic sequence so a
# single strided AP covers the whole group.
_ADJ = {0: [1, 3, 5, 7], 1: [0, 2], 2: [1], 3: [0, 4], 4: [3],
        5: [0, 6], 6: [5], 7: [0, 8], 8: [7]}
EDGES = [(d, d) for d in range(NN)]
SRC_GROUPS = []     # (src, [dsts], edge_base)
for _s in range(NN):
    SRC_GROUPS.append((_s, _ADJ[_s], len(EDGES)))
    for _d in _ADJ[_s]:
        EDGES.append((_s, _d))
NE = len(EDGES)     # 25
assert NE == 25
# in-edges per destination (edge indices into EDGES), self-loop first
IN_EDGES = [[e for e, (_s, d) in enumerate(EDGES) if d == dd and _s == dd] +
            [e for e, (_s, d) in enumerate(EDGES) if d == dd and _s != dd]
            for dd in range(NN)]


class Cfg:
    # engine assignment cyclers (per op class)
    add_engines = ("gpsimd", "vector")
    leaky_engines = ("scalar", "vector")
    agg_engines = ("vector",)                # fused STT agg (AP scalar)
    aggi_engines = ("scalar",)               # agg init: copy with scale
    pcopy_engines = ("scalar", "vector")   # proj psum->sbuf copies (no PSUM on gpsimd)
    gcopy_engines = ("scalar", "vector")     # gm psum->sbuf copies
    # prev-block PE closures drained after each proj / gm psum unit
    fill_proj = (0, 0, 2, 2, 2, 2, 3, 3, 3, 3,
                 3, 3, 3, 3, 3, 3, 3, 3, 2, 2)
    fill_gm = 2


def build_program(cfg: Cfg):
    nc = bacc.Bacc("TRN2", target_bir_lowering=False, debug=False)

    def eng(name):
        return {"vector": nc.vector, "gpsimd": nc.gpsimd,
                "scalar": nc.scalar}[name]

    def copy_op(ename, dst_ap, src_ap):
        if ename == "scalar":
            nc.scalar.copy(dst_ap, src_ap)
        else:
            eng(ename).tensor_copy(dst_ap, src_ap)

    def leaky_op(ename, dst_ap, src_ap):
        if ename == "scalar":
            nc.scalar.activation(dst_ap, src_ap,
                                 mybir.ActivationFunctionType.Prelu,
                                 alpha=NEG_SLOPE)
        else:
            eng(ename).scalar_tensor_tensor(
                dst_ap, src_ap, NEG_SLOPE, src_ap,
                op0=mybir.AluOpType.mult, op1=mybir.AluOpType.max)

    # DRAM tensors
    xT_d = nc.dram_tensor("xT", [C, NBLK * NGT], BF16, kind="ExternalInput")
    wl_d = nc.dram_tensor("wl", [C, C], BF16, kind="ExternalInput")
    wr_d = nc.dram_tensor("wr", [C, C], BF16, kind="ExternalInput")
    attbl_d = nc.dram_tensor("attbl", [128, 2 * NE * NE], FP8,
                             kind="ExternalInput")
    smat_d = nc.dram_tensor("smat", [NE, NE], F32, kind="ExternalInput")
    identf_d = nc.dram_tensor("identf", [128, 128], F32, kind="ExternalInput")
    out_d = nc.dram_tensor("out", [BC, NN * C], BF16, kind="ExternalOutput")

    with tile.TileContext(nc) as tc:
        with (
            tc.tile_pool(name="const", bufs=1) as cpool,
            tc.tile_pool(name="xin", bufs=2) as xpool,
            tc.tile_pool(name="proj", bufs=2) as prpool,
            tc.tile_pool(name="edge", bufs=2) as epool,
            tc.tile_pool(name="soft", bufs=2) as spool,
            tc.tile_pool(name="gm", bufs=2) as gmpool,
            tc.tile_pool(name="outp", bufs=4) as opool,
            tc.tile_pool(name="ps_proj", bufs=2, space="PSUM") as ps_proj,
            tc.tile_pool(name="ps_gm", bufs=2, space="PSUM") as ps_gm,
            tc.tile_pool(name="ps_sc", bufs=2, space="PSUM") as ps_sc,
        ):
            # ---- constants ----
            wl_sb = cpool.tile([128, 2 * C], BF16, tag="wl")
            wr_sb = cpool.tile([128, 2 * C], BF16, tag="wr")
            nc.sync.dma_start(wl_sb[:, 0:C], wl_d[0:128, :])
            nc.sync.dma_start(wl_sb[:, C:2 * C], wl_d[128:256, :])
            nc.sync.dma_start(wr_sb[:, 0:C], wr_d[0:128, :])
            nc.sync.dma_start(wr_sb[:, C:2 * C], wr_d[128:256, :])
            attbl_sb = cpool.tile([128, 2 * NE * NE], FP8, tag="attbl")
            nc.sync.dma_start(attbl_sb[:], attbl_d[:])
            smat_sb = cpool.tile([NE, NE], F32, tag="smat")
            nc.sync.dma_start(smat_sb[:], smat_d[:])
            identf_sb = cpool.tile([128, 128], F32, tag="identf")
            nc.sync.dma_start(identf_sb[:], identf_d[:])

            ecyc = {"add": 0, "leaky": 0, "agg": 0, "aggi": 0,
                    "pcopy": 0, "gcopy": 0}

            def cyc(kind):
                lst = getattr(cfg, kind + "_engines")
                e = lst[ecyc[kind] % len(lst)]
                ecyc[kind] += 1
                return e

            def emit_edge_ops(pv):
                """DVE-class ops producing this block's leaky tiles (st,
                fp8).  The adds write st directly; leaky is applied in
                place.  Consumed by next iteration's score matmuls."""
                xlT, xrT = pv["xlT"], pv["xrT"]
                st = pv["st"]
                for dch in range(2):
                    for e, (s, d) in enumerate(EDGES):
                        h = epool.tile([128, G], BF16, tag=f"h{dch}_{e%4}")
                        eng(cyc("add")).tensor_tensor(
                            h[:], xlT[dch][:, s * G:(s + 1) * G],
                            xrT[dch][:, d * G:(d + 1) * G],
                            op=mybir.AluOpType.add)
                        leaky_op(cyc("leaky"),
                                 st[dch][:, e * G:(e + 1) * G], h[:])

            def make_fill(pv):
                """PE + small closures for prev block's score/softmax path."""
                fill = []
                st = pv["st"]
                sc_ps = ps_sc.tile([NE, G], F32, tag="sc")

                def mk_sc(e, dch):
                    def f():
                        blk = (dch * NE + e) * NE
                        nc.tensor.matmul(
                            sc_ps[:], attbl_sb[:, blk:blk + NE],
                            st[dch][:, e * G:(e + 1) * G],
                            start=(e == 0 and dch == 0),
                            stop=(e == NE - 1 and dch == 1))
                    return f

                for e in range(NE):
                    for dch in range(2):
                        fill.append(mk_sc(e, dch))

                ex_sb = spool.tile([NE, G], F32, tag="ex")
                den_sb = spool.tile([NE, G], F32, tag="den")
                alphaT = spool.tile([128, NT * NE], F32, tag="alphaT")
                pv["alphaT"] = alphaT

                def f_exp():
                    nc.scalar.activation(ex_sb[:], sc_ps[:],
                                         mybir.ActivationFunctionType.Exp)
                fill.append(f_exp)

                mis1 = ps_sc.tile([128, max(G, 2 * NT * NE)], F32, tag="mis")
                den_ps = mis1[0:NE, 0:G]

                def f_den():
                    nc.tensor.matmul(den_ps, smat_sb[:], ex_sb[:],
                                     start=True, stop=True)
                fill.append(f_den)

                def f_denc():
                    nc.scalar.copy(den_sb[:], den_ps)
                fill.append(f_denc)

                exT_ps = ps_sc.tile([128, max(G, 2 * NT * NE)], F32, tag="mis")
                dT0 = NT * NE

                def f_tr():
                    for t in range(NT):
                        nc.tensor.transpose(
                            exT_ps[:, t * NE:(t + 1) * NE],
                            ex_sb[:, t * 128:(t + 1) * 128],
                            identf_sb[0:NE, 0:NE])
                        nc.tensor.transpose(
                            exT_ps[:, dT0 + t * NE:dT0 + (t + 1) * NE],
                            den_sb[:, t * 128:(t + 1) * 128],
                            identf_sb[0:NE, 0:NE])
                fill.append(f_tr)

                def f_alpha():
                    rdenT = spool.tile([128, NT * NE], F32, tag="rdenT")
                    nc.vector.reciprocal(rdenT[:],
                                         exT_ps[:, dT0:dT0 + dT0])
                    nc.vector.tensor_tensor(
                        alphaT[:], exT_ps[:, 0:dT0], rdenT[:],
                        op=mybir.AluOpType.mult)
                fill.append(f_alpha)
                return fill

            def emit_agg(pv):
                """Aggregation + output DMA for prev block (late in queues)."""
                alphaT = pv["alphaT"]
                xl_gms = pv["xl_gms"]
                bb = pv["b"]
                for t in range(NT):
                    xl_gm = xl_gms[t]
                    out_t = opool.tile([128, NN * C], BF16, tag="out_t")
                    for d in range(NN):
                        es = IN_EDGES[d]
                        e0 = es[0]
                        ie = cyc("aggi")
                        if ie == "scalar":
                            nc.scalar.activation(
                                out_t[:, d * C:(d + 1) * C],
                                xl_gm[:, d * C:(d + 1) * C],
                                mybir.ActivationFunctionType.Copy,
                                scale=alphaT[:, t * NE + e0:t * NE + e0 + 1])
                        else:
                            eng(ie).tensor_scalar_mul(
                                out_t[:, d * C:(d + 1) * C],
                                xl_gm[:, d * C:(d + 1) * C],
                                alphaT[:, t * NE + e0:t * NE + e0 + 1])
                        for e in es[1:]:
                            s = EDGES[e][0]
                            eng(cyc("agg")).scalar_tensor_tensor(
                                out_t[:, d * C:(d + 1) * C],
                                xl_gm[:, s * C:(s + 1) * C],
                                alphaT[:, t * NE + e:t * NE + e + 1],
                                out_t[:, d * C:(d + 1) * C],
                                op0=mybir.AluOpType.mult,
                                op1=mybir.AluOpType.add)
                    nc.sync.dma_start(
                        out_d[bb * G + t * 128:bb * G + (t + 1) * 128, :],
                        out_t[:])

            prev = None       # state dict of block b-1
            for b in range(NBLK + 1):
                cur = None
                fill = make_fill(prev) if prev is not None else []
                fi = 0

                def drain(k):
                    nonlocal fi
                    for _ in range(k):
                        if fi < len(fill):
                            fill[fi]()
                            fi += 1

                if b < NBLK:
                    cur = {"b": b}
                    # ---- load xT block ----
                    xt = []
                    for chk in range(2):
                        t_ = xpool.tile([128, NGT], BF16, tag=f"xt{chk}")
                        nc.sync.dma_start(
                            t_[:], xT_d[chk * 128:(chk + 1) * 128,
                                        b * NGT:(b + 1) * NGT])
                        xt.append(t_)

                    # ---- channel-major projections ----
                    # xlT/xrT: [128, 9, G] per (w-matrix, out-chunk)
                    xlT = [prpool.tile([128, NN * G], BF16, tag=f"xlT{d}",
                                       name=f"xlT{d}") for d in range(2)]
                    xrT = [prpool.tile([128, NN * G], BF16, tag=f"xrT{d}",
                                       name=f"xrT{d}") for d in range(2)]
                    cur["xlT"], cur["xrT"] = xlT, xrT
                    cur["st"] = [epool.tile([128, NE * G], FP8,
                                            tag=f"st{d}", name=f"st{d}")
                                 for d in range(2)]
                    ui = 0
                    for (wsb, dest) in ((wl_sb, xlT), (wr_sb, xrT)):
                        for dch in range(2):
                            s = 0
                            while s < NN:
                                npair = min(2, NN - s)
                                ps = ps_proj.tile([128, 2 * G], F32,
                                                  tag="ps_proj")
                                for j in range(npair):
                                    for kc in range(2):
                                        nc.tensor.matmul(
                                            ps[:, j * G:(j + 1) * G],
                                            wsb[:, kc * C + dch * 128:
                                                kc * C + dch * 128 + 128],
                                            xt[kc][:, (s + j) * G:
                                                   (s + j + 1) * G],
                                            start=(kc == 0), stop=(kc == 1))
                                copy_op(cyc("pcopy"),
                                        dest[dch][:, s * G:(s + npair) * G],
                                        ps[:, 0:npair * G])
                                drain(cfg.fill_proj[ui]
                                      if ui < len(cfg.fill_proj) else 2)
                                ui += 1
                                s += npair

                    # this block's edge ops (consumed by next iteration)
                    emit_edge_ops(cur)

                    # ---- graph-major xl projection ----
                    xl_gms = [gmpool.tile([128, NN * C], BF16,
                                          tag=f"xl_gm{t}", name=f"xl_gm{t}")
                              for t in range(NT)]
                    cur["xl_gms"] = xl_gms
                    for t in range(NT):
                        s = 0
                        while s < NN:
                            npair = min(2, NN - s)
                            ps = ps_gm.tile([128, 512], F32, tag="ps_gm")
                            for j in range(npair):
                                for kc in range(2):
                                    nc.tensor.matmul(
                                        ps[:, j * C:(j + 1) * C],
                                        xt[kc][:, (s + j) * G + t * 128:
                                               (s + j) * G + (t + 1) * 128],
                                        wl_sb[:, kc * C:(kc + 1) * C],
                                        start=(kc == 0), stop=(kc == 1))
                            copy_op(cyc("gcopy"),
                                    xl_gms[t][:, s * C:(s + npair) * C],
                                    ps[:, 0:npair * C])
                            drain(cfg.fill_gm)
                            s += npair

                drain(len(fill))      # flush remaining prev closures
                if prev is not None:
                    emit_agg(prev)
                prev = cur

    nc.compile()
    return nc


def make_host_inputs(x, W_l, W_r, att, cfg: Cfg):
    """Builds the per-core input maps (host-side sharding + layout prep)."""
    x = np.asarray(x, dtype=np.float32)
    W_l = np.ascontiguousarray(np.asarray(W_l, dtype=np.float32))
    W_r = np.ascontiguousarray(np.asarray(W_r, dtype=np.float32))
    att = np.asarray(att, dtype=np.float32)
    bf = ml_dtypes.bfloat16
    f8 = ml_dtypes.bfloat16

    # att (x) onehot(e) stationary blocks for the leaky-relu'd score dot
    attbl = np.zeros((128, 2, NE, NE), dtype=np.float32)
    for dch in range(2):
        for e in range(NE):
            attbl[:, dch, e, e] = att[dch * 128:(dch + 1) * 128]
    attbl = attbl.reshape(128, 2 * NE * NE).astype(f8)

    smat = np.zeros((NE, NE), dtype=np.float32)
    for e1, (_s1, d1) in enumerate(EDGES):
        for e2, (_s2, d2) in enumerate(EDGES):
            if d1 == d2:
                smat[e1, e2] = 1.0

    ident = np.eye(128, dtype=np.float32)

    in_maps = []
    for c in range(N_CORES):
        xc = x[c * BC:(c + 1) * BC]                       # [BC, 9, 256]
        xT = np.ascontiguousarray(
            xc.reshape(NBLK, G, NN, C).transpose(3, 0, 2, 1).reshape(
                C, NBLK * NGT).astype(bf))
        in_maps.append({
            "xT": xT,
            "wl": W_l.astype(bf),
            "wr": W_r.astype(bf),
            "attbl": attbl,
            "smat": smat,
            "identf": ident,
        })
    return in_maps


_CACHE = {}


def _cfg_key(cfg: Cfg):
    return (cfg.add_engines, cfg.leaky_engines, cfg.agg_engines,
            cfg.aggi_engines, cfg.pcopy_engines, cfg.gcopy_engines,
            cfg.fill_proj, cfg.fill_gm)


def _get_program(cfg: Cfg):
    key = _cfg_key(cfg)
    if key not in _CACHE:
        _CACHE[key] = build_program(cfg)
    return _CACHE[key]


def kernel(x, W_l, W_r, att, bias, cfg: Cfg = None, trace: bool = False,
           _results_holder: dict = None, **run_kwargs):
    cfg = cfg or Cfg()
    nc = _get_program(cfg)
    in_maps = make_host_inputs(x, W_l, W_r, att, cfg)
    res = run_bass_kernel_spmd(nc, in_maps, core_ids=list(range(N_CORES)),
                               trace=trace, **run_kwargs)
    if _results_holder is not None:
        _results_holder["res"] = res
    outs = [np.asarray(r["out"], dtype=np.float32).reshape(BC, NN, C)
            for r in res.results]
    out = np.concatenate(outs, axis=0)
    bias = np.asarray(bias, dtype=np.float32)
    if np.any(bias):
        out = out + bias
    return out.astype(np.float32)


# revision 26
# speedup vs baseline: 1.0489x; 1.0489x over previous
"""GATv2Conv batched-graph kernel for Trainium2 (8 NeuronCores, data-parallel).

Problem: B=16384 independent 9-node graphs, C_in=C_out=256, fixed edge list
(16 directed tree edges + 9 self-loops = 25 edges), GATv2 attention.

V3 design (vs the 342us baseline):
  - Custom fused DVE op LEAKY_ADD_ANT: st = max(xl+xr, 0.2*(xl+xr)) in ONE
    ~1 cyc/elem instruction (the stock path was a tensor_tensor add plus a
    2.2 cyc/elem scalar_tensor_tensor or an ACT Prelu).  Registered into
    the ant custom-DVE table at import time.
  - Edges ordered self-loops-first then grouped by source node so the
    leaky-adds run as a few wide strided ops ([128, 9*G] for all
    self-loops at once; per-src groups use a uniform dst stride).
  - Shallow cross-block pipeline: PE per block runs proj -> gm-proj ->
    scores -> den/transposes with no idle gaps (score inputs are produced
    by the DVE-class engines during the proj/gm phases), which lets the
    tensor engine ramp to its full 2.4 GHz p-state.  The aggregation of
    block b-1 (DVE/ACT) is drained through block b's proj loop.
  - Aggregation in graph-major via fused mult-add chains (alpha is a
    per-partition scalar there), STT or the AFFINE_THEN_ADD custom op.
  - bias handled host-side (zeros in this problem).
"""

import sys

if "/opt/trn_rl_repo" not in sys.path:
    sys.path.insert(0, "/opt/trn_rl_repo")

import numpy as np
import ml_dtypes

import concourse.bass as bass
import concourse.bacc as bacc
import concourse.mybir as mybir
from concourse import tile
from concourse.bass_utils import run_bass_kernel_spmd

# ---- register the fused leaky-add custom DVE op ----
from concourse import dve_ops as _dops
from concourse.dve_spec import Spec as _Spec, Src0 as _S0, Src1 as _S1, \
    C0 as _C0, maxx as _maxx, lower as _lower
from concourse.dve_uop import DveOpSpec as _DveOpSpec

_LSPEC = _Spec(
    body=_maxx(_S0 + _S1, (_S0 + _S1) * _C0),
    reference=lambda in0, in1, s0, s1, imm2: np.maximum(
        in0 + in1, (in0 + in1) * s0),
)


def _register_leaky_add():
    if "LEAKY_ADD_ANT" in _dops._SUB_OPCODE_FOR_NAME:
        return next(op for op in _dops.OPS if op.name == "LEAKY_ADD_ANT")
    op = _dops.DveOp("LEAKY_ADD_ANT", _LSPEC, subdim=False, uops_sha={})
    for ver in ("v3", "v4"):
        try:
            sha = _DveOpSpec(
                name="LEAKY_ADD_ANT", opcode=0,
                uops=_lower(_LSPEC, ver=ver), rd1_en=True).sha(ver)
            op.uops_sha[ver] = sha
        except Exception:
            pass
    row = _dops._CUSTOM_DVE_ROW_BASE + len(_dops.OPS)
    assert row < 0x20
    _dops.OPS.append(op)
    _dops.CUSTOM_DVE_SPECS["LEAKY_ADD_ANT"] = _LSPEC
    _dops._SUB_OPCODE_FOR_NAME["LEAKY_ADD_ANT"] = row
    return op


LEAKY_ADD_ANT = _register_leaky_add()

F32 = mybir.dt.float32
BF16 = mybir.dt.bfloat16

N_CORES = 8
B_TOTAL = 16384
NEG_SLOPE = 0.2
BC = B_TOTAL // N_CORES          # graphs per core
NN = 9                           # nodes per graph
C = 256                          # channels
G = 512                          # graphs per block
NBLK = BC // G                   # blocks per core
NT = G // 128                    # 128-graph subtiles per block
NGT = NN * G                     # columns per (chunk, block)

# ---- static edge list ----
# Order: 9 self-loops first (edge e = node e), then tree edges grouped by
# SOURCE node (dst lists are arithmetic sequences -> one strided AP each).
_ADJ = {0: [1, 3, 5, 7], 1: [0, 2], 2: [1], 3: [0, 4], 4: [3],
        5: [0, 6], 6: [5], 7: [0, 8], 8: [7]}
EDGES = [(d, d) for d in range(NN)]
SRC_GROUPS = []     # (src, [dsts], edge_base)
for _s in range(NN):
    SRC_GROUPS.append((_s, _ADJ[_s], len(EDGES)))
    for _d in _ADJ[_s]:
        EDGES.append((_s, _d))
NE = len(EDGES)     # 25
assert NE == 25
IN_EDGES = [[e for e, (_s, d) in enumerate(EDGES) if d == dd and _s == dd] +
            [e for e, (_s, d) in enumerate(EDGES) if d == dd and _s != dd]
            for dd in range(NN)]

# leaky-add units: (kind, src, dsts, edge_base); "sl" covers all self-loops
LADD_UNITS = [("sl", None, None, 0)] + \
    [("grp", s, dsts, eb) for (s, dsts, eb) in SRC_GROUPS]


class Cfg:
    # per leaky-add unit (cycled): "dve" = fused custom op on DVE;
    # "pa" = tensor_tensor add on Pool + Prelu on ACT (per-edge 2-D ops)
    ladd_units = ("dve", "pa", "dve", "pa", "dve", "pa", "dve", "pa",
                  "dve", "pa")
    agg_engines = ("affine",)          # "vector" (STT) | "affine" (custom)
    aggi_engines = ("scalar",)         # "scalar" | "affine" (zero trick)
    pcopy_engines = ("scalar", "vector")
    gcopy_engines = ("scalar", "scalar", "vector")
    prev_per_unit = 2                  # prev-block agg closures per psum unit


def build_program(cfg: Cfg):
    nc = bacc.Bacc("TRN2", target_bir_lowering=False, debug=False)

    def eng(name):
        return {"vector": nc.vector, "gpsimd": nc.gpsimd,
                "scalar": nc.scalar}[name]

    def copy_op(ename, dst_ap, src_ap):
        if ename == "scalar":
            nc.scalar.copy(dst_ap, src_ap)
        else:
            eng(ename).tensor_copy(dst_ap, src_ap)

    # DRAM tensors
    xT_d = nc.dram_tensor("xT", [C, NBLK * NGT], BF16, kind="ExternalInput")
    wl_d = nc.dram_tensor("wl", [C, C], BF16, kind="ExternalInput")
    wr_d = nc.dram_tensor("wr", [C, C], BF16, kind="ExternalInput")
    attbl_d = nc.dram_tensor("attbl", [128, 2 * NE * NE], BF16,
                             kind="ExternalInput")
    smat_d = nc.dram_tensor("smat", [NE, NE], F32, kind="ExternalInput")
    identf_d = nc.dram_tensor("identf", [128, 128], F32, kind="ExternalInput")
    out_d = nc.dram_tensor("out", [BC, NN * C], BF16, kind="ExternalOutput")

    with tile.TileContext(nc) as tc:
        with (
            tc.tile_pool(name="const", bufs=1) as cpool,
            tc.tile_pool(name="xin", bufs=2) as xpool,
            tc.tile_pool(name="proj", bufs=1) as prpool,
            tc.tile_pool(name="edge", bufs=1) as epool,
            tc.tile_pool(name="soft", bufs=2) as spool,
            tc.tile_pool(name="gm", bufs=2) as gmpool,
            tc.tile_pool(name="outp", bufs=4) as opool,
            tc.tile_pool(name="ps_proj", bufs=2, space="PSUM") as ps_proj,
            tc.tile_pool(name="ps_gm", bufs=2, space="PSUM") as ps_gm,
            tc.tile_pool(name="ps_sc", bufs=1, space="PSUM") as ps_sc,
        ):
            # ---- constants ----
            wl_sb = cpool.tile([128, 2 * C], BF16, tag="wl")
            wr_sb = cpool.tile([128, 2 * C], BF16, tag="wr")
            nc.sync.dma_start(wl_sb[:, 0:C], wl_d[0:128, :])
            nc.sync.dma_start(wl_sb[:, C:2 * C], wl_d[128:256, :])
            nc.sync.dma_start(wr_sb[:, 0:C], wr_d[0:128, :])
            nc.sync.dma_start(wr_sb[:, C:2 * C], wr_d[128:256, :])
            attbl_sb = cpool.tile([128, 2 * NE * NE], BF16, tag="attbl")
            nc.sync.dma_start(attbl_sb[:], attbl_d[:])
            smat_sb = cpool.tile([NE, NE], F32, tag="smat")
            nc.sync.dma_start(smat_sb[:], smat_d[:])
            identf_sb = cpool.tile([128, 128], F32, tag="identf")
            nc.sync.dma_start(identf_sb[:], identf_d[:])
            zero_sb = cpool.tile([128, C], BF16, tag="zero")
            nc.vector.memset(zero_sb[:], 0.0)

            ecyc = {}

            def cyc(kind):
                lst = getattr(cfg, kind + "_engines")
                i = ecyc.get(kind, 0)
                ecyc[kind] = i + 1
                return lst[i % len(lst)]

            def emit_edge_ops(pv):
                """Fused leaky(xl+xr) producing this block's st tiles."""
                xlT, xrT = pv["xlT"], pv["xrT"]
                st = pv["st"]
                ui = 0
                for dch in range(2):
                    for (kind, s, dsts, eb) in LADD_UNITS:
                        how = cfg.ladd_units[ui % len(cfg.ladd_units)]
                        ui += 1
                        if kind == "sl":
                            edges_2d = [(d, d, d) for d in range(NN)]
                            in0 = xlT[dch][:]
                            in1 = xrT[dch][:]
                            dst = st[dch][:, 0:NN, :]
                            n = NN
                        else:
                            n = len(dsts)
                            edges_2d = [(s, d, eb + i)
                                        for i, d in enumerate(dsts)]
                            if n == 1:
                                in1 = xrT[dch][:, dsts[0]:dsts[0] + 1, :]
                            else:
                                step = dsts[1] - dsts[0]
                                in1 = xrT[dch][:, dsts[0]:dsts[-1] + 1:step,
                                               :]
                            in0 = xlT[dch][:, s:s + 1, :].broadcast_to(
                                [128, n, G])
                            dst = st[dch][:, eb:eb + n, :]
                        if how == "dve":
                            nc.vector._custom_dve(
                                LEAKY_ADD_ANT, out=dst, in0=in0, in1=in1,
                                s0=NEG_SLOPE)
                        else:
                            # per-edge 2-D ops: Pool add + ACT Prelu
                            for (ss, dd, ee) in edges_2d:
                                dst2 = st[dch][:, ee, :]
                                nc.gpsimd.tensor_tensor(
                                    dst2, xlT[dch][:, ss, :],
                                    xrT[dch][:, dd, :],
                                    op=mybir.AluOpType.add)
                                nc.scalar.activation(
                                    dst2, dst2,
                                    mybir.ActivationFunctionType.Prelu,
                                    alpha=NEG_SLOPE)

            def make_agg_closures(pv):
                """Aggregation + output DMA closures for prev block."""
                alphaT = pv["alphaT"]
                xl_gms = pv["xl_gms"]
                bb = pv["b"]
                work = []
                for t in range(NT):
                    xl_gm = xl_gms[t]
                    out_t = opool.tile([128, NN * C], BF16, tag="out_t")

                    def mk(d, t=t, xl_gm=xl_gm, out_t=out_t):
                        def f():
                            es = IN_EDGES[d]
                            e0 = es[0]
                            ie = cyc("aggi")
                            al0 = alphaT[:, t * NE + e0:t * NE + e0 + 1]
                            if ie == "scalar":
                                nc.scalar.activation(
                                    out_t[:, d * C:(d + 1) * C],
                                    xl_gm[:, d * C:(d + 1) * C],
                                    mybir.ActivationFunctionType.Copy,
                                    scale=al0)
                            else:
                                nc.vector.affine_then_add(
                                    out_t[:, d * C:(d + 1) * C],
                                    xl_gm[:, d * C:(d + 1) * C],
                                    zero_sb[:], al0, 0.0)
                            for e in es[1:]:
                                s = EDGES[e][0]
                                al = alphaT[:, t * NE + e:t * NE + e + 1]
                                ge = cyc("agg")
                                if ge == "affine":
                                    nc.vector.affine_then_add(
                                        out_t[:, d * C:(d + 1) * C],
                                        xl_gm[:, s * C:(s + 1) * C],
                                        out_t[:, d * C:(d + 1) * C],
                                        al, 0.0)
                                else:
                                    nc.vector.scalar_tensor_tensor(
                                        out_t[:, d * C:(d + 1) * C],
                                        xl_gm[:, s * C:(s + 1) * C], al,
                                        out_t[:, d * C:(d + 1) * C],
                                        op0=mybir.AluOpType.mult,
                                        op1=mybir.AluOpType.add)
                        return f
                    for d in range(NN):
                        work.append(mk(d))

                    def dma(t=t, out_t=out_t):
                        nc.sync.dma_start(
                            out_d[bb * G + t * 128:
                                  bb * G + (t + 1) * 128, :],
                            out_t[:])
                    work.append(dma)
                return work

            prev = None
            for b in range(NBLK + 1):
                cur = None
                pwork = make_agg_closures(prev) if prev is not None else []
                pi = 0

                def drain(k):
                    nonlocal pi
                    for _ in range(k):
                        if pi < len(pwork):
                            pwork[pi]()
                            pi += 1

                if b < NBLK:
                    cur = {"b": b}
                    xt = []
                    for chk in range(2):
                        t_ = xpool.tile([128, NGT], BF16, tag=f"xt{chk}")
                        nc.sync.dma_start(
                            t_[:], xT_d[chk * 128:(chk + 1) * 128,
                                        b * NGT:(b + 1) * NGT])
                        xt.append(t_)

                    # ---- channel-major projections ----
                    xlT = [prpool.tile([128, NN, G], BF16, tag=f"xlT{d}",
                                       name=f"xlT{d}") for d in range(2)]
                    xrT = [prpool.tile([128, NN, G], BF16, tag=f"xrT{d}",
                                       name=f"xrT{d}") for d in range(2)]
                    cur["xlT"], cur["xrT"] = xlT, xrT
                    cur["st"] = [epool.tile([128, NE, G], BF16,
                                            tag=f"st{d}", name=f"st{d}")
                                 for d in range(2)]
                    for (wsb, dest) in ((wl_sb, xlT), (wr_sb, xrT)):
                        for dch in range(2):
                            s = 0
                            while s < NN:
                                npair = min(2, NN - s)
                                ps = ps_proj.tile([128, 2, G], F32,
                                                  tag="ps_proj")
                                for j in range(npair):
                                    for kc in range(2):
                                        nc.tensor.matmul(
                                            ps[:, j, :],
                                            wsb[:, kc * C + dch * 128:
                                                kc * C + dch * 128 + 128],
                                            xt[kc][:, (s + j) * G:
                                                   (s + j + 1) * G],
                                            start=(kc == 0), stop=(kc == 1))
                                copy_op(cyc("pcopy"),
                                        dest[dch][:, s:s + npair, :],
                                        ps[:, 0:npair, :])
                                drain(cfg.prev_per_unit)
                                s += npair

                    # this block's leaky-adds (run during proj/gm phases)
                    emit_edge_ops(cur)

                    # ---- graph-major xl projection ----
                    xl_gms = [gmpool.tile([128, NN * C], BF16,
                                          tag=f"xl_gm{t}", name=f"xl_gm{t}")
                              for t in range(NT)]
                    cur["xl_gms"] = xl_gms
                    for t in range(NT):
                        s = 0
                        while s < NN:
                            npair = min(2, NN - s)
                            ps = ps_gm.tile([128, 512], F32, tag="ps_gm")
                            for j in range(npair):
                                for kc in range(2):
                                    nc.tensor.matmul(
                                        ps[:, j * C:(j + 1) * C],
                                        xt[kc][:, (s + j) * G + t * 128:
                                               (s + j) * G + (t + 1) * 128],
                                        wl_sb[:, kc * C:(kc + 1) * C],
                                        start=(kc == 0), stop=(kc == 1))
                            copy_op(cyc("gcopy"),
                                    xl_gms[t][:, s * C:(s + npair) * C],
                                    ps[:, 0:npair * C])
                            drain(cfg.prev_per_unit)
                            s += npair

                    drain(len(pwork))

                    # ---- scores + softmax for THIS block ----
                    st = cur["st"]
                    sc_ps = ps_sc.tile([NE, G], F32, tag="sc")
                    for e in range(NE):
                        for dch in range(2):
                            blk = (dch * NE + e) * NE
                            nc.tensor.matmul(
                                sc_ps[:], attbl_sb[:, blk:blk + NE],
                                st[dch][:, e, :],
                                start=(e == 0 and dch == 0),
                                stop=(e == NE - 1 and dch == 1))
                    ex_sb = spool.tile([NE, G], F32, tag="ex")
                    nc.scalar.activation(ex_sb[:], sc_ps[:],
                                         mybir.ActivationFunctionType.Exp)
                    mis1 = ps_sc.tile([128, G], F32, tag="mis")
                    den_ps = mis1[0:NE, 0:G]
                    nc.tensor.matmul(den_ps, smat_sb[:], ex_sb[:],
                                     start=True, stop=True)
                    den_sb = spool.tile([NE, G], F32, tag="den")
                    nc.scalar.copy(den_sb[:], den_ps)
                    exT_ps = ps_sc.tile([128, G], F32, tag="mis")
                    dT0 = NT * NE
                    for t in range(NT):
                        nc.tensor.transpose(
                            exT_ps[:, t * NE:(t + 1) * NE],
                            ex_sb[:, t * 128:(t + 1) * 128],
                            identf_sb[0:NE, 0:NE])
                        nc.tensor.transpose(
                            exT_ps[:, dT0 + t * NE:dT0 + (t + 1) * NE],
                            den_sb[:, t * 128:(t + 1) * 128],
                            identf_sb[0:NE, 0:NE])
                    rdenT = spool.tile([128, NT * NE], F32, tag="rdenT")
                    nc.vector.reciprocal(rdenT[:],
                                         exT_ps[:, dT0:dT0 + dT0])
                    alphaT = spool.tile([128, NT * NE], F32, tag="alphaT")
                    nc.vector.tensor_tensor(
                        alphaT[:], exT_ps[:, 0:dT0], rdenT[:],
                        op=mybir.AluOpType.mult)
                    cur["alphaT"] = alphaT
                else:
                    drain(len(pwork))

                prev = cur

    nc.compile()
    return nc


def make_host_inputs(x, W_l, W_r, att, cfg: Cfg):
    """Builds the per-core input maps (host-side sharding + layout prep)."""
    x = np.asarray(x, dtype=np.float32)
    W_l = np.ascontiguousarray(np.asarray(W_l, dtype=np.float32))
    W_r = np.ascontiguousarray(np.asarray(W_r, dtype=np.float32))
    att = np.asarray(att, dtype=np.float32)
    bf = ml_dtypes.bfloat16

    attbl = np.zeros((128, 2, NE, NE), dtype=np.float32)
    for dch in range(2):
        for e in range(NE):
            attbl[:, dch, e, e] = att[dch * 128:(dch + 1) * 128]
    attbl = attbl.reshape(128, 2 * NE * NE).astype(bf)

    smat = np.zeros((NE, NE), dtype=np.float32)
    for e1, (_s1, d1) in enumerate(EDGES):
        for e2, (_s2, d2) in enumerate(EDGES):
            if d1 == d2:
                smat[e1, e2] = 1.0

    ident = np.eye(128, dtype=np.float32)

    in_maps = []
    for c in range(N_CORES):
        xc = x[c * BC:(c + 1) * BC]                       # [BC, 9, 256]
        xT = np.ascontiguousarray(
            xc.reshape(NBLK, G, NN, C).transpose(3, 0, 2, 1).reshape(
                C, NBLK * NGT).astype(bf))
        in_maps.append({
            "xT": xT,
            "wl": W_l.astype(bf),
            "wr": W_r.astype(bf),
            "attbl": attbl,
            "smat": smat,
            "identf": ident,
        })
    return in_maps


_CACHE = {}


def _cfg_key(cfg: Cfg):
    return (cfg.ladd_units, cfg.agg_engines, cfg.aggi_engines,
            cfg.pcopy_engines, cfg.gcopy_engines, cfg.prev_per_unit)


def _get_program(cfg: Cfg):
    key = _cfg_key(cfg)
    if key not in _CACHE:
        _CACHE[key] = build_program(cfg)
    return _CACHE[key]


def kernel(x, W_l, W_r, att, bias, cfg: Cfg = None, trace: bool = False,
           _results_holder: dict = None, **run_kwargs):
    cfg = cfg or Cfg()
    nc = _get_program(cfg)
    in_maps = make_host_inputs(x, W_l, W_r, att, cfg)
    res = run_bass_kernel_spmd(nc, in_maps, core_ids=list(range(N_CORES)),
                               trace=trace, **run_kwargs)
    if _results_holder is not None:
        _results_holder["res"] = res
    outs = [np.asarray(r["out"], dtype=np.float32).reshape(BC, NN, C)
            for r in res.results]
    out = np.concatenate(outs, axis=0)
    bias = np.asarray(bias, dtype=np.float32)
    if np.any(bias):
        out = out + bias
    return out.astype(np.float32)


# revision 27
# speedup vs baseline: 1.1247x; 1.0723x over previous
"""GATv2Conv batched-graph kernel for Trainium2 (8 NeuronCores, data-parallel).

Problem: B=16384 independent 9-node graphs, C_in=C_out=256, fixed edge list
(16 directed tree edges + 9 self-loops = 25 edges), GATv2 attention.

V3 design (vs the 342us baseline):
  - Custom fused DVE op LEAKY_ADD_ANT: st = max(xl+xr, 0.2*(xl+xr)) in ONE
    ~1 cyc/elem instruction (the stock path was a tensor_tensor add plus a
    2.2 cyc/elem scalar_tensor_tensor or an ACT Prelu).  Registered into
    the ant custom-DVE table at import time.
  - Edges ordered self-loops-first then grouped by source node so the
    leaky-adds run as a few wide strided ops ([128, 9*G] for all
    self-loops at once; per-src groups use a uniform dst stride).
  - Shallow cross-block pipeline: PE per block runs proj -> gm-proj ->
    scores -> den/transposes with no idle gaps (score inputs are produced
    by the DVE-class engines during the proj/gm phases), which lets the
    tensor engine ramp to its full 2.4 GHz p-state.  The aggregation of
    block b-1 (DVE/ACT) is drained through block b's proj loop.
  - Aggregation in graph-major via fused mult-add chains (alpha is a
    per-partition scalar there), STT or the AFFINE_THEN_ADD custom op.
  - bias handled host-side (zeros in this problem).
"""

import sys

if "/opt/trn_rl_repo" not in sys.path:
    sys.path.insert(0, "/opt/trn_rl_repo")

import numpy as np
import ml_dtypes

import concourse.bass as bass
import concourse.bacc as bacc
import concourse.mybir as mybir
from concourse import tile
from concourse.bass_utils import run_bass_kernel_spmd

# ---- register the fused leaky-add custom DVE op ----
from concourse import dve_ops as _dops
from concourse.dve_spec import Spec as _Spec, Src0 as _S0, Src1 as _S1, \
    C0 as _C0, maxx as _maxx, lower as _lower
from concourse.dve_uop import DveOpSpec as _DveOpSpec

_LSPEC = _Spec(
    body=_maxx(_S0 + _S1, (_S0 + _S1) * _C0),
    reference=lambda in0, in1, s0, s1, imm2: np.maximum(
        in0 + in1, (in0 + in1) * s0),
)


def _register_leaky_add():
    if "LEAKY_ADD_ANT" in _dops._SUB_OPCODE_FOR_NAME:
        return next(op for op in _dops.OPS if op.name == "LEAKY_ADD_ANT")
    op = _dops.DveOp("LEAKY_ADD_ANT", _LSPEC, subdim=False, uops_sha={})
    for ver in ("v3", "v4"):
        try:
            sha = _DveOpSpec(
                name="LEAKY_ADD_ANT", opcode=0,
                uops=_lower(_LSPEC, ver=ver), rd1_en=True).sha(ver)
            op.uops_sha[ver] = sha
        except Exception:
            pass
    row = _dops._CUSTOM_DVE_ROW_BASE + len(_dops.OPS)
    assert row < 0x20
    _dops.OPS.append(op)
    _dops.CUSTOM_DVE_SPECS["LEAKY_ADD_ANT"] = _LSPEC
    _dops._SUB_OPCODE_FOR_NAME["LEAKY_ADD_ANT"] = row
    return op


LEAKY_ADD_ANT = _register_leaky_add()

F32 = mybir.dt.float32
BF16 = mybir.dt.bfloat16

N_CORES = 8
B_TOTAL = 16384
NEG_SLOPE = 0.2
BC = B_TOTAL // N_CORES          # graphs per core
NN = 9                           # nodes per graph
C = 256                          # channels
G = 512                          # graphs per block
NBLK = BC // G                   # blocks per core
NT = G // 128                    # 128-graph subtiles per block
NGT = NN * G                     # columns per (chunk, block)

# ---- static edge list ----
# Order: 9 self-loops first (edge e = node e), then tree edges grouped by
# SOURCE node (dst lists are arithmetic sequences -> one strided AP each).
_ADJ = {0: [1, 3, 5, 7], 1: [0, 2], 2: [1], 3: [0, 4], 4: [3],
        5: [0, 6], 6: [5], 7: [0, 8], 8: [7]}
EDGES = [(d, d) for d in range(NN)]
SRC_GROUPS = []     # (src, [dsts], edge_base)
for _s in range(NN):
    SRC_GROUPS.append((_s, _ADJ[_s], len(EDGES)))
    for _d in _ADJ[_s]:
        EDGES.append((_s, _d))
NE = len(EDGES)     # 25
assert NE == 25
IN_EDGES = [[e for e, (_s, d) in enumerate(EDGES) if d == dd and _s == dd] +
            [e for e, (_s, d) in enumerate(EDGES) if d == dd and _s != dd]
            for dd in range(NN)]

# leaky-add units: (kind, src, dsts, edge_base); "sl" covers all self-loops
LADD_UNITS = [("sl", None, None, 0)] + \
    [("grp", s, dsts, eb) for (s, dsts, eb) in SRC_GROUPS]


class Cfg:
    # per leaky-add unit (cycled): "dve" = fused custom op on DVE;
    # "pa" = tensor_tensor add on Pool + Prelu on ACT (per-edge 2-D ops)
    ladd_units = ("dve", "pa", "dve", "pa", "dve", "pa", "dve", "pa",
                  "dve", "pa")
    agg_engines = ("affine",)          # "vector" (STT) | "affine" (custom)
    aggi_engines = ("scalar",)         # "scalar" | "affine" (zero trick)
    pcopy_engines = ("scalar", "vector")
    gcopy_engines = ("scalar", "scalar", "vector")
    prev_per_unit = 2                  # prev-block agg closures per psum unit


def build_program(cfg: Cfg):
    nc = bacc.Bacc("TRN2", target_bir_lowering=False, debug=False)

    def eng(name):
        return {"vector": nc.vector, "gpsimd": nc.gpsimd,
                "scalar": nc.scalar}[name]

    def copy_op(ename, dst_ap, src_ap):
        if ename == "scalar":
            nc.scalar.copy(dst_ap, src_ap)
        else:
            eng(ename).tensor_copy(dst_ap, src_ap)

    # DRAM tensors
    xT_d = nc.dram_tensor("xT", [C, NBLK * NGT], BF16, kind="ExternalInput")
    wl_d = nc.dram_tensor("wl", [C, C], BF16, kind="ExternalInput")
    wr_d = nc.dram_tensor("wr", [C, C], BF16, kind="ExternalInput")
    attbl_d = nc.dram_tensor("attbl", [128, 2 * NE * NE], BF16,
                             kind="ExternalInput")
    smat_d = nc.dram_tensor("smat", [NE, NE], F32, kind="ExternalInput")
    identf_d = nc.dram_tensor("identf", [128, 128], F32, kind="ExternalInput")
    out_d = nc.dram_tensor("out", [BC, NN * C], BF16, kind="ExternalOutput")

    with tile.TileContext(nc) as tc:
        with (
            tc.tile_pool(name="const", bufs=1) as cpool,
            tc.tile_pool(name="xin", bufs=2) as xpool,
            tc.tile_pool(name="proj", bufs=1) as prpool,
            tc.tile_pool(name="edge", bufs=1) as epool,
            tc.tile_pool(name="soft", bufs=2) as spool,
            tc.tile_pool(name="gm", bufs=2) as gmpool,
            tc.tile_pool(name="outp", bufs=4) as opool,
            tc.tile_pool(name="ps_proj", bufs=2, space="PSUM") as ps_proj,
            tc.tile_pool(name="ps_gm", bufs=2, space="PSUM") as ps_gm,
            tc.tile_pool(name="ps_sc", bufs=1, space="PSUM") as ps_sc,
        ):
            # ---- constants ----
            wl_sb = cpool.tile([128, 2 * C], BF16, tag="wl")
            wr_sb = cpool.tile([128, 2 * C], BF16, tag="wr")
            nc.sync.dma_start(wl_sb[:, 0:C], wl_d[0:128, :])
            nc.sync.dma_start(wl_sb[:, C:2 * C], wl_d[128:256, :])
            nc.sync.dma_start(wr_sb[:, 0:C], wr_d[0:128, :])
            nc.sync.dma_start(wr_sb[:, C:2 * C], wr_d[128:256, :])
            attbl_sb = cpool.tile([128, 2 * NE * NE], BF16, tag="attbl")
            nc.sync.dma_start(attbl_sb[:], attbl_d[:])
            smat_sb = cpool.tile([NE, NE], F32, tag="smat")
            nc.sync.dma_start(smat_sb[:], smat_d[:])
            identf_sb = cpool.tile([128, 128], F32, tag="identf")
            nc.sync.dma_start(identf_sb[:], identf_d[:])
            zero_sb = cpool.tile([128, C], BF16, tag="zero")
            nc.vector.memset(zero_sb[:], 0.0)

            ecyc = {}

            def cyc(kind):
                lst = getattr(cfg, kind + "_engines")
                i = ecyc.get(kind, 0)
                ecyc[kind] = i + 1
                return lst[i % len(lst)]

            def emit_edge_ops(pv, dch):
                """Fused leaky(xl+xr) producing this block's st tiles."""
                xlT, xrT = pv["xlT"], pv["xrT"]
                st = pv["st"]
                if True:
                    for ui, (kind, s, dsts, eb) in enumerate(LADD_UNITS):
                        how = cfg.ladd_units[ui % len(cfg.ladd_units)]
                        if kind == "sl":
                            edges_2d = [(d, d, d) for d in range(NN)]
                            in0 = xlT[dch][:]
                            in1 = xrT[dch][:]
                            dst = st[dch][:, 0:NN, :]
                            n = NN
                        else:
                            n = len(dsts)
                            edges_2d = [(s, d, eb + i)
                                        for i, d in enumerate(dsts)]
                            if n == 1:
                                in1 = xrT[dch][:, dsts[0]:dsts[0] + 1, :]
                            else:
                                step = dsts[1] - dsts[0]
                                in1 = xrT[dch][:, dsts[0]:dsts[-1] + 1:step,
                                               :]
                            in0 = xlT[dch][:, s:s + 1, :].broadcast_to(
                                [128, n, G])
                            dst = st[dch][:, eb:eb + n, :]
                        if how == "dve":
                            nc.vector._custom_dve(
                                LEAKY_ADD_ANT, out=dst, in0=in0, in1=in1,
                                s0=NEG_SLOPE)
                        else:
                            # per-edge 2-D ops: Pool add + ACT Prelu
                            for (ss, dd, ee) in edges_2d:
                                dst2 = st[dch][:, ee, :]
                                nc.gpsimd.tensor_tensor(
                                    dst2, xlT[dch][:, ss, :],
                                    xrT[dch][:, dd, :],
                                    op=mybir.AluOpType.add)
                                nc.scalar.activation(
                                    dst2, dst2,
                                    mybir.ActivationFunctionType.Prelu,
                                    alpha=NEG_SLOPE)

            def make_agg_closures(pv):
                """Aggregation + output DMA closures for prev block."""
                alphaT = pv["alphaT"]
                xl_gms = pv["xl_gms"]
                bb = pv["b"]
                work = []
                for t in range(NT):
                    xl_gm = xl_gms[t]
                    out_t = opool.tile([128, NN * C], BF16, tag="out_t")

                    def mk_op(d, i, t=t, xl_gm=xl_gm, out_t=out_t):
                        def f():
                            es = IN_EDGES[d]
                            e = es[i]
                            al = alphaT[:, t * NE + e:t * NE + e + 1]
                            dsl = out_t[:, d * C:(d + 1) * C]
                            ssl = xl_gm[:, EDGES[e][0] * C:
                                        (EDGES[e][0] + 1) * C]
                            if i == 0:
                                ie = cyc("aggi")
                                if ie == "scalar":
                                    nc.scalar.activation(
                                        dsl, ssl,
                                        mybir.ActivationFunctionType.Copy,
                                        scale=al)
                                else:
                                    nc.vector.affine_then_add(
                                        dsl, ssl, zero_sb[:], al, 0.0)
                            else:
                                ge = cyc("agg")
                                if ge == "affine":
                                    nc.vector.affine_then_add(
                                        dsl, ssl, dsl, al, 0.0)
                                else:
                                    nc.vector.scalar_tensor_tensor(
                                        dsl, ssl, al, dsl,
                                        op0=mybir.AluOpType.mult,
                                        op1=mybir.AluOpType.add)
                        return f
                    # round-robin across dsts so consecutive DVE ops hit
                    # different out_t regions (no RMW pipeline stalls)
                    maxdeg = max(len(es) for es in IN_EDGES)
                    for i in range(maxdeg):
                        for d in range(NN):
                            if i < len(IN_EDGES[d]):
                                work.append(mk_op(d, i))

                    def dma(t=t, out_t=out_t):
                        nc.sync.dma_start(
                            out_d[bb * G + t * 128:
                                  bb * G + (t + 1) * 128, :],
                            out_t[:])
                    work.append(dma)
                return work

            prev = None
            for b in range(NBLK + 1):
                cur = None
                pwork = make_agg_closures(prev) if prev is not None else []
                pi = 0

                def drain(k):
                    nonlocal pi
                    for _ in range(k):
                        if pi < len(pwork):
                            pwork[pi]()
                            pi += 1

                if b < NBLK:
                    cur = {"b": b}
                    xt = []
                    for chk in range(2):
                        t_ = xpool.tile([128, NGT], BF16, tag=f"xt{chk}")
                        nc.sync.dma_start(
                            t_[:], xT_d[chk * 128:(chk + 1) * 128,
                                        b * NGT:(b + 1) * NGT])
                        xt.append(t_)

                    # ---- channel-major projections ----
                    xlT = [prpool.tile([128, NN, G], BF16, tag=f"xlT{d}",
                                       name=f"xlT{d}") for d in range(2)]
                    xrT = [prpool.tile([128, NN, G], BF16, tag=f"xrT{d}",
                                       name=f"xrT{d}") for d in range(2)]
                    cur["xlT"], cur["xrT"] = xlT, xrT
                    cur["st"] = [epool.tile([128, NE, G], BF16,
                                            tag=f"st{d}", name=f"st{d}")
                                 for d in range(2)]
                    for (wsb, dest, dch) in ((wl_sb, xlT, 0),
                                             (wr_sb, xrT, 0),
                                             (wl_sb, xlT, 1),
                                             (wr_sb, xrT, 1)):
                        if True:
                            s = 0
                            while s < NN:
                                npair = min(2, NN - s)
                                ps = ps_proj.tile([128, 2, G], F32,
                                                  tag="ps_proj")
                                for j in range(npair):
                                    for kc in range(2):
                                        nc.tensor.matmul(
                                            ps[:, j, :],
                                            wsb[:, kc * C + dch * 128:
                                                kc * C + dch * 128 + 128],
                                            xt[kc][:, (s + j) * G:
                                                   (s + j + 1) * G],
                                            start=(kc == 0), stop=(kc == 1))
                                copy_op(cyc("pcopy"),
                                        dest[dch][:, s:s + npair, :],
                                        ps[:, 0:npair, :])
                                drain(cfg.prev_per_unit)
                                s += npair
                        if wsb is wr_sb:
                            emit_edge_ops(cur, dch)

                    # ---- graph-major xl projection ----
                    xl_gms = [gmpool.tile([128, NN * C], BF16,
                                          tag=f"xl_gm{t}", name=f"xl_gm{t}")
                              for t in range(NT)]
                    cur["xl_gms"] = xl_gms
                    for t in range(NT):
                        s = 0
                        while s < NN:
                            npair = min(2, NN - s)
                            ps = ps_gm.tile([128, 512], F32, tag="ps_gm")
                            for j in range(npair):
                                for kc in range(2):
                                    nc.tensor.matmul(
                                        ps[:, j * C:(j + 1) * C],
                                        xt[kc][:, (s + j) * G + t * 128:
                                               (s + j) * G + (t + 1) * 128],
                                        wl_sb[:, kc * C:(kc + 1) * C],
                                        start=(kc == 0), stop=(kc == 1))
                            copy_op(cyc("gcopy"),
                                    xl_gms[t][:, s * C:(s + npair) * C],
                                    ps[:, 0:npair * C])
                            drain(cfg.prev_per_unit)
                            s += npair

                    drain(len(pwork))

                    # ---- scores + softmax for THIS block ----
                    st = cur["st"]
                    sc_ps = ps_sc.tile([NE, G], F32, tag="sc")
                    for e in range(NE):
                        for dch in range(2):
                            blk = (dch * NE + e) * NE
                            nc.tensor.matmul(
                                sc_ps[:], attbl_sb[:, blk:blk + NE],
                                st[dch][:, e, :],
                                start=(e == 0 and dch == 0),
                                stop=(e == NE - 1 and dch == 1))
                    ex_sb = spool.tile([NE, G], F32, tag="ex")
                    nc.scalar.activation(ex_sb[:], sc_ps[:],
                                         mybir.ActivationFunctionType.Exp)
                    mis1 = ps_sc.tile([128, G], F32, tag="mis")
                    den_ps = mis1[0:NE, 0:G]
                    nc.tensor.matmul(den_ps, smat_sb[:], ex_sb[:],
                                     start=True, stop=True)
                    den_sb = spool.tile([NE, G], F32, tag="den")
                    nc.scalar.copy(den_sb[:], den_ps)
                    exT_ps = ps_sc.tile([128, G], F32, tag="mis")
                    dT0 = NT * NE
                    for t in range(NT):
                        nc.tensor.transpose(
                            exT_ps[:, t * NE:(t + 1) * NE],
                            ex_sb[:, t * 128:(t + 1) * 128],
                            identf_sb[0:NE, 0:NE])
                        nc.tensor.transpose(
                            exT_ps[:, dT0 + t * NE:dT0 + (t + 1) * NE],
                            den_sb[:, t * 128:(t + 1) * 128],
                            identf_sb[0:NE, 0:NE])
                    rdenT = spool.tile([128, NT * NE], F32, tag="rdenT")
                    nc.vector.reciprocal(rdenT[:],
                                         exT_ps[:, dT0:dT0 + dT0])
                    alphaT = spool.tile([128, NT * NE], F32, tag="alphaT")
                    nc.vector.tensor_tensor(
                        alphaT[:], exT_ps[:, 0:dT0], rdenT[:],
                        op=mybir.AluOpType.mult)
                    cur["alphaT"] = alphaT
                else:
                    drain(len(pwork))

                prev = cur

    nc.compile()
    return nc


def make_host_inputs(x, W_l, W_r, att, cfg: Cfg):
    """Builds the per-core input maps (host-side sharding + layout prep)."""
    x = np.asarray(x, dtype=np.float32)
    W_l = np.ascontiguousarray(np.asarray(W_l, dtype=np.float32))
    W_r = np.ascontiguousarray(np.asarray(W_r, dtype=np.float32))
    att = np.asarray(att, dtype=np.float32)
    bf = ml_dtypes.bfloat16

    attbl = np.zeros((128, 2, NE, NE), dtype=np.float32)
    for dch in range(2):
        for e in range(NE):
            attbl[:, dch, e, e] = att[dch * 128:(dch + 1) * 128]
    attbl = attbl.reshape(128, 2 * NE * NE).astype(bf)

    smat = np.zeros((NE, NE), dtype=np.float32)
    for e1, (_s1, d1) in enumerate(EDGES):
        for e2, (_s2, d2) in enumerate(EDGES):
            if d1 == d2:
                smat[e1, e2] = 1.0

    ident = np.eye(128, dtype=np.float32)

    in_maps = []
    for c in range(N_CORES):
        xc = x[c * BC:(c + 1) * BC]                       # [BC, 9, 256]
        xT = np.ascontiguousarray(
            xc.reshape(NBLK, G, NN, C).transpose(3, 0, 2, 1).reshape(
                C, NBLK * NGT).astype(bf))
        in_maps.append({
            "xT": xT,
            "wl": W_l.astype(bf),
            "wr": W_r.astype(bf),
            "attbl": attbl,
            "smat": smat,
            "identf": ident,
        })
    return in_maps


_CACHE = {}


def _cfg_key(cfg: Cfg):
    return (cfg.ladd_units, cfg.agg_engines, cfg.aggi_engines,
            cfg.pcopy_engines, cfg.gcopy_engines, cfg.prev_per_unit)


def _get_program(cfg: Cfg):
    key = _cfg_key(cfg)
    if key not in _CACHE:
        _CACHE[key] = build_program(cfg)
    return _CACHE[key]


def kernel(x, W_l, W_r, att, bias, cfg: Cfg = None, trace: bool = False,
           _results_holder: dict = None, **run_kwargs):
    cfg = cfg or Cfg()
    nc = _get_program(cfg)
    in_maps = make_host_inputs(x, W_l, W_r, att, cfg)
    res = run_bass_kernel_spmd(nc, in_maps, core_ids=list(range(N_CORES)),
                               trace=trace, **run_kwargs)
    if _results_holder is not None:
        _results_holder["res"] = res
    outs = [np.asarray(r["out"], dtype=np.float32).reshape(BC, NN, C)
            for r in res.results]
    out = np.concatenate(outs, axis=0)
    bias = np.asarray(bias, dtype=np.float32)
    if np.any(bias):
        out = out + bias
    return out.astype(np.float32)
